# revision 2
# baseline (speedup 1.0000x reference)
"""BoxFilter (radius r, clipped window, no normalization) on 8 Trainium2
NeuronCores.

out = diff_y(diff_x(x.cumsum(H), r).cumsum(W), r)  ==  for each (b, c) plane:
out[h, w] = sum over the clipped (2r+1)x(2r+1) window of x.

Strategy (data-parallel over batch, 4 batches/core):
  - Host: shard x over batch, cast to fp16.
  - H-axis filter on TensorE: banded-ones matmul (contraction over the
    partition dim) with small halo matmuls accumulating the chunk-boundary
    contributions into the same PSUM bank (fp32 accumulation).
  - PSUM -> SBUF (fp16) copy on ScalarE.
  - W-axis filter in a single VectorE tensor_tensor_scan per tile using the
    recurrence state = (v[w+r] + state) - v[w-r-1] over a zero-padded row,
    which yields the full clipped sliding-window sum in one pass.
  - DMA fp16 result back; host casts to fp32 and reassembles.
"""

import numpy as np

N = 512          # plane height/width
P = 128          # SBUF partitions
NCH = N // P     # 4 row chunks per plane
B, C = 32, 8
NCORES = 8
BPC = B // NCORES          # batches per core
NPLANES = BPC * C          # planes per core

_cache = {}


def _make_weights(r):
    k = np.arange(P)[:, None]
    m = np.arange(P)[None, :]
    D = (np.abs(k - m) <= r).astype(np.float16)
    kp = np.arange(r)[:, None]
    mp = np.arange(P)[None, :]
    # prev-chunk halo rows (x rows c*P-r .. c*P-1) feed outputs m' = 0..r-1
    Wp = ((mp < r) & (kp >= mp)).astype(np.float16)
    # next-chunk halo rows (x rows (c+1)*P .. +r-1) feed outputs m' = P-r..P-1
    Wn = ((mp >= P - r) & ((mp - (P - r)) >= kp)).astype(np.float16)
    Wfull = np.concatenate([Wp, Wn], axis=0)
    return D, Wfull, Wn, Wp


def _build(r):
    import concourse.bacc as bacc
    import concourse.mybir as mybir
    import concourse.tile as tile

    F16 = mybir.dt.float16
    L = 2 * r + 1  # left pad (zeros) ahead of each row for the scan

    nc = bacc.Bacc("TRN2", debug=False, enable_asserts=False)
    x = nc.dram_tensor("x", [NPLANES, N, N], F16, kind="ExternalInput").ap()
    d_in = nc.dram_tensor("d", [P, P], F16, kind="ExternalInput").ap()
    wfull_in = nc.dram_tensor("wfull", [2 * r, P], F16, kind="ExternalInput").ap()
    wn_in = nc.dram_tensor("wn", [r, P], F16, kind="ExternalInput").ap()
    wp_in = nc.dram_tensor("wp", [r, P], F16, kind="ExternalInput").ap()
    y = nc.dram_tensor("y", [NPLANES, N, N], F16, kind="ExternalOutput").ap()

    with tile.TileContext(nc) as tc:
        with tc.tile_pool(name="wts", bufs=1) as wts, \
             tc.tile_pool(name="xp", bufs=4) as xp, \
             tc.tile_pool(name="hp", bufs=4) as hp, \
             tc.tile_pool(name="sp", bufs=1) as sp, \
             tc.tile_pool(name="op", bufs=4) as op, \
             tc.tile_pool(name="ps", bufs=8, space="PSUM") as ps:

            D = wts.tile([P, P], F16, tag="D")
            Wfull = wts.tile([2 * r, P], F16, tag="Wfull")
            Wn = wts.tile([r, P], F16, tag="Wn")
            Wp = wts.tile([r, P], F16, tag="Wp")
            nc.sync.dma_start(D[:, :], d_in[:, :])
            nc.sync.dma_start(Wfull[:, :], wfull_in[:, :])
            nc.sync.dma_start(Wn[:, :], wn_in[:, :])
            nc.sync.dma_start(Wp[:, :], wp_in[:, :])

            # fixed scan-input tiles; pad columns stay zero across reuse
            S_bufs = []
            for i in range(4):
                S = sp.tile([P, L + N + r], F16, tag=f"S{i}")
                nc.vector.memset(S[:, 0:L], 0.0)
                nc.vector.memset(S[:, L + N:], 0.0)
                S_bufs.append(S)

            u = 0
            for p in range(NPLANES):
                X_tiles = []
                for c in range(NCH):
                    X = xp.tile([P, N], F16, tag="X")
                    nc.sync.dma_start(X[:, :], x[p, c * P:(c + 1) * P, :])
                    X_tiles.append(X)
                psums = []
                for c in range(NCH):
                    psum = ps.tile([P, N], mybir.dt.float32, tag="psum")
                    nc.tensor.matmul(psum[:, :], D[:, :], X_tiles[c][:, :],
                                     start=True, stop=False)
                    psums.append(psum)
                for c in range(NCH):
                    if c == 0:
                        H = hp.tile([r, N], F16, tag="H")
                        nc.sync.dma_start(H[:, :], x[p, P:P + r, :])
                        nc.tensor.matmul(psums[c][:, :], Wn[:, :], H[:, :],
                                         start=False, stop=True)
                    elif c == NCH - 1:
                        H = hp.tile([r, N], F16, tag="H")
                        nc.sync.dma_start(H[:, :], x[p, c * P - r:c * P, :])
                        nc.tensor.matmul(psums[c][:, :], Wp[:, :], H[:, :],
                                         start=False, stop=True)
                    else:
                        H = hp.tile([2 * r, N], F16, tag="H")
                        nc.sync.dma_start(H[0:r, :], x[p, c * P - r:c * P, :])
                        nc.sync.dma_start(H[r:2 * r, :],
                                          x[p, (c + 1) * P:(c + 1) * P + r, :])
                        nc.tensor.matmul(psums[c][:, :], Wfull[:, :], H[:, :],
                                         start=False, stop=True)
                for c in range(NCH):
                    S = S_bufs[u % 4]
                    u += 1
                    nc.scalar.copy(S[:, L:L + N], psums[c][:, :])
                    O = op.tile([P, N + r], F16, tag="O")
                    nc.vector.tensor_tensor_scan(
                        O[:, :],
                        S[:, L:],
                        S[:, 0:N + r],
                        0.0,
                        mybir.AluOpType.add,
                        mybir.AluOpType.subtract,
                    )
                    nc.sync.dma_start(y[p, c * P:(c + 1) * P, :], O[:, r:])

    nc.compile()
    return nc


def kernel(x: np.ndarray, r) -> np.ndarray:
    from concourse import bass_utils

    r = int(np.asarray(r))
    assert x.shape == (B, C, N, N), x.shape
    assert 1 <= r < P, r

    if r not in _cache:
        _cache[r] = _build(r)
    nc = _cache[r]

    D, Wfull, Wn, Wp = _make_weights(r)
    x16 = x.astype(np.float16)
    in_maps = []
    for core in range(NCORES):
        shard = np.ascontiguousarray(
            x16[core * BPC:(core + 1) * BPC].reshape(NPLANES, N, N))
        in_maps.append({
            "x": shard,
            "d": D,
            "wfull": Wfull,
            "wn": Wn,
            "wp": Wp,
        })

    res = bass_utils.run_bass_kernel_spmd(nc, in_maps, core_ids=list(range(NCORES)))
    out = np.concatenate(
        [res.results[c]["y"].reshape(BPC, C, N, N) for c in range(NCORES)], axis=0)
    kernel.last_exec_time_ns = res.exec_time_ns
    kernel.last_results = res
    return out.astype(np.float32)
